# revision 21
# baseline (speedup 1.0000x reference)
"""CNLP (common-neighbor link prediction) kernel for Trainium2, 8 NeuronCores.

Reference computation (per query edge e = (i, j)):
    cn  = adj[i] * adj[j]                      # common-neighbor indicator [N]
    xcn = cn @ x                               # sum of common-neighbor feats
    xij = relu(x[i]*x[j] @ Wa.T + ba) @ Wb.T + bb
    hcn = (relu->relu->lin) 3-layer MLP on xcn
    out = (hcn * beta + xij) @ Wl.T + bl       # [E, 1]

Sharding: edges (E=8192) split 8 x 1024 across cores; adj/x/weights replicated.

Device strategy per core (v2 — bitpacked adjacency + fp8 DoubleRow):
  - adj rows are BITPACKED on the host: [N, 1280] bytes (8 nodes/byte,
    little bit order).  gpsimd dma_gather(transpose=True) pulls the two
    packed rows per edge batch (128 edges) — 8x less HBM traffic than fp8.
  - DVE ANDs the packed rows (bitwise AND == product for binary rows),
    then extracts 8 bit-planes with one fused shift+mask tensor_scalar op
    per plane: each surviving bit sits at position 4 of its byte, i.e.
    fp8e4m3 value 2^-5; the matching x-table rows are pre-scaled by 2^5.
  - x is uploaded as a [10240, 512] fp8 table of [x_hi | x_lo] (hi/lo split
    of 32*x, so two e4m3 matmuls recover ~9-bit-mantissa accuracy), rows
    permuted to plane-major order so each (plane, byte-chunk) pair of
    128-node groups is contiguous.
  - TensorE runs fp8 DoubleRow matmuls (256-deep contraction per
    instruction, 0.5 PE cycles/row) with the x-table pairs as the
    STATIONARY operand (contiguous, satisfies the dual-fp8 Ldweights
    stride rules) and the cn planes as the moving operand: the output
    lands feature-major ([d, e] = xcn^T) so no PE transposes are needed;
    ACT+DVE fold the hi/lo halves while converting to bf16.
  - xi/xj are gathered from a separate natural-order bf16 [x_hi | x_lo]
    table (exact same path as v1, proven on HW).
  - MLPs run in bf16 (1 PE cycle/row), feature-major, biases+ReLU fused
    into ScalarE activations.  Output [1, 1024] fp32 per core.
"""

import numpy as np
import ml_dtypes

import concourse.bacc as bacc
import concourse.tile as tile
import concourse.mybir as mybir
from concourse.bass_utils import run_bass_kernel_spmd

BF16 = mybir.dt.bfloat16
FP32 = mybir.dt.float32
F32R = mybir.dt.float32r
FP8 = mybir.dt.float8e4
I16 = mybir.dt.int16
AF = mybir.ActivationFunctionType
ALU = mybir.AluOpType
DR = mybir.MatmulPerfMode.DoubleRow
FP8_NP = ml_dtypes.float8_e4m3

N_CORES = 8
N, E, D, H = 10000, 8192, 256, 256
NPAD = 10240                      # n padded to a multiple of 2048
PB = NPAD // 8                    # packed bytes per adjacency row (1280)
C2 = PB // 256                    # 256-byte chunks per row (5)
NPAIR = 8 * C2                    # DoubleRow matmuls per edge batch (40)
EC = E // N_CORES                 # 1024 edges per core
EB = 128                          # edges per batch (one gather tile)
XSCALE = 32.0                     # x table pre-scale (2^5, exact in fp8)
NHALF = 1                         # 1 = single fp8 x table, 2 = [hi | lo]
XW = D * NHALF                    # fp8 x-table row width
NCH = XW // 128                   # output column chunks (2 or 4)


def build_program(npad=NPAD, ec=EC):
    nb = ec // EB                  # edge batches (8)
    ehw = min(512, ec)             # edge columns per MLP matmul
    n_eh = ec // ehw               # MLP halves (2)
    n_xs = 8                       # x table sub-tiles
    prs = NPAIR // n_xs            # pairs per x sub-tile (5)
    u16 = PB // 2                  # int16 lanes per packed row (640)

    nc = bacc.Bacc("TRN2", target_bir_lowering=False, debug=False,
                   enable_asserts=False, num_devices=N_CORES)

    adjp_d = nc.dram_tensor("adjp", [N, PB], FP8, kind="ExternalInput")
    xhl_d = nc.dram_tensor("xhl", [128, npad // 128, XW], FP8,
                           kind="ExternalInput")
    xg_d = nc.dram_tensor("xg", [npad, 2 * D], BF16, kind="ExternalInput")
    # gather indices: raw node ids (adjacency rows + natural-order xg rows)
    idx0_d = nc.dram_tensor("idx0", [128, ec // 16], I16, kind="ExternalInput")
    idx1_d = nc.dram_tensor("idx1", [128, ec // 16], I16, kind="ExternalInput")
    # all MLP weights pre-transposed to [partition, k-chunk, h] and packed
    # into one tensor (one contiguous descriptor per partition); biases+beta
    # likewise packed as fp32
    wpack_d = nc.dram_tensor("wpack", [128, 5 * 2 * H + 2], BF16,
                             kind="ExternalInput")
    bpack_d = nc.dram_tensor("bpack", [128, 11], FP32, kind="ExternalInput")
    bl_d = nc.dram_tensor("bl", [1, 1], FP32, kind="ExternalInput")
    out_d = nc.dram_tensor("out", [1, ec], FP32, kind="ExternalOutput")

    with tile.TileContext(nc) as tc:
        with (
            tc.tile_pool(name="const", bufs=1) as constp,
            tc.tile_pool(name="gath", bufs=1) as gathp,
            tc.tile_pool(name="plane", bufs=2) as planep,
            tc.tile_pool(name="work", bufs=2) as workp,
            tc.tile_pool(name="ghl", bufs=1) as ghlp,
            tc.tile_pool(name="acts", bufs=4) as actp,
            tc.tile_pool(name="px", bufs=3 - NHALF, space="PSUM") as pxp,
            tc.tile_pool(name="pm", bufs=2, space="PSUM") as pmp,
            tc.tile_pool(name="po", bufs=2, space="PSUM") as pop,
        ):
            # ---- persistent loads -------------------------------------
            # idx tiles FIRST: every gather waits on them, and HWDGE DMAs
            # execute in FIFO order per issuing engine
            idx0_sb = constp.tile([128, ec // 16], I16)
            nc.sync.dma_start(idx0_sb[:], idx0_d[:])
            idx1_sb = constp.tile([128, ec // 16], I16)
            nc.sync.dma_start(idx1_sb[:], idx1_d[:])

            # whole x table in one DMA (pre-transposed on the host: one
            # contiguous 20KB descriptor per partition)
            xall = constp.tile([128, NPAIR * 2, XW], FP8)
            nc.sync.dma_start(xall[:], xhl_d[:])
            xhl_t = [xall[:, i * 2 * prs:(i + 1) * 2 * prs, :]
                     for i in range(n_xs)]

            wpack = constp.tile([128, 5 * 2 * H + 2], BF16)
            nc.sync.dma_start(wpack[:], wpack_d[:])
            w_sb = {nm: wpack[:, i * 2 * H:(i + 1) * 2 * H]
                    .rearrange("p (k h) -> p k h", k=2)
                    for i, nm in enumerate(("wat", "wbt", "w1t", "w2t", "w3t"))}
            wlt_sb = wpack[:, 10 * H:10 * H + 2].rearrange(
                "p (k o) -> p k o", k=2)
            bpack = constp.tile([128, 11], FP32)
            nc.sync.dma_start(bpack[:], bpack_d[:])
            b_sb = {nm: bpack[:, i * 2:(i + 1) * 2]
                    .rearrange("p (t o) -> p t o", t=2)
                    for i, nm in enumerate(("ba", "bb", "b1", "b2", "b3"))}
            beta_sb = bpack[:, 10:11]
            bl_sb = constp.tile([1, 1], FP32)
            nc.sync.dma_start(bl_sb[:], bl_d[:])

            # hoist every gather: descriptor generation serializes on the
            # gpsimd engine, so issue all eight back-to-back up front
            gt = {}
            for s in range(n_eh):
                isl = slice(s * (ehw // 16), (s + 1) * (ehw // 16))
                for nm, isb in (("g0", idx0_sb), ("g1", idx1_sb)):
                    g = gathp.tile([128, PB // 128, ehw], FP8,
                                   tag=f"{nm}_{s}")
                    nc.gpsimd.dma_gather(g[:], adjp_d[:], isb[:, isl],
                                         ehw, ehw, elem_size=PB,
                                         transpose=True)
                    gt[nm, s] = g
            ghl_t = {}
            for s in range(n_eh):
                isl = slice(s * (ehw // 16), (s + 1) * (ehw // 16))
                for nm, isb in (("xi", idx0_sb), ("xj", idx1_sb)):
                    ghl = ghlp.tile([128, 4, ehw], BF16, tag=f"ghl{nm}{s}")
                    nc.gpsimd.dma_gather(ghl[:], xg_d[:], isb[:, isl],
                                         ehw, ehw, elem_size=2 * D,
                                         transpose=True)
                    ghl_t[nm, s] = ghl

            xcnT = constp.tile([128, 2, ec], BF16)   # feature-major xcn
            out_sb = constp.tile([1, ec], FP32)

            # ---- MLP for one edge-half (ehw edges), feature-major f32r;
            # emitted mid-loop so its PE work overlaps gather-bound spans.
            def lin_h(src_ap, wname, bname, relu, dst):
                w, bias = w_sb[wname], b_sb[bname]
                for t in range(2):
                    pm = pmp.tile([128, ehw], FP32, tag="pm")
                    for k in range(2):
                        nc.tensor.matmul(
                            pm[:], w[:, k, t * 128:(t + 1) * 128],
                            src_ap[:, k, :], start=(k == 0), stop=(k == 1))
                    dsl = dst[:, t, :]
                    if t % 2 == 0:
                        nc.scalar.activation(
                            dsl, pm[:], AF.Relu if relu else AF.Identity,
                            bias=bias[:, t, :])
                    elif relu:
                        nc.vector.tensor_scalar(
                            dsl, pm[:], bias[:, t, :], 0.0, ALU.add, ALU.max)
                    else:
                        nc.vector.tensor_scalar_add(dsl, pm[:], bias[:, t, :])
                return dst

            def mlp_half(hh):
                esl = slice(hh * ehw, (hh + 1) * ehw)
                xiT = actp.tile([128, 2, ehw], FP32, tag="act")
                xjT = actp.tile([128, 2, ehw], FP32, tag="act")
                for dst, nm in ((xiT, "xi"), (xjT, "xj")):
                    ghl = ghl_t[nm, hh]
                    nc.vector.tensor_add(dst[:], ghl[:, 0:2, :], ghl[:, 2:4, :])
                pT = actp.tile([128, 2, ehw], BF16, tag="actb")
                nc.vector.tensor_mul(pT[:], xiT[:], xjT[:])
                u = lin_h(pT[:], "wat", "ba", True,
                          actp.tile([128, 2, ehw], BF16, tag="actb",
                                    name=f"u{hh}"))
                xijT = lin_h(u[:], "wbt", "bb", False,
                             actp.tile([128, 2, ehw], BF16, tag="actb",
                                       name=f"xijT{hh}"))
                h = xcnT[:, :, esl]
                for li, (wn, bn, rl) in enumerate((
                        ("w1t", "b1", True), ("w2t", "b2", True),
                        ("w3t", "b3", False))):
                    h = lin_h(h, wn, bn, rl,
                              actp.tile([128, 2, ehw], BF16, tag="actb",
                                        name=f"h{hh}_{li}"))[:]
                nc.vector.tensor_add(h, h, xijT[:])
                po = pop.tile([1, ehw], FP32, tag="po")
                for k in range(2):
                    nc.tensor.matmul(po[:], wlt_sb[:, k, :], h[:, k, :],
                                     start=(k == 0), stop=(k == 1))
                osl = slice(hh * ehw, (hh + 1) * ehw)
                nc.scalar.activation(out_sb[:, osl], po[:], AF.Identity,
                                     bias=bl_sb[:])
                nc.sync.dma_start(out_d[:, osl], out_sb[:, osl])

            # ---- main loop: packed gather -> AND -> planes -> xcn^T ---
            for sb in range(n_eh):
                g0, g1 = gt["g0", sb], gt["g1", sb]
                # cn = g0 AND g1 on int16 lanes (two packed bytes per lane)
                v0 = g0[:].rearrange("p a b -> p (a b)").bitcast(I16)
                v1 = g1[:].rearrange("p a b -> p (a b)").bitcast(I16)
                usz = PB * ehw // 256
                nc.vector.tensor_tensor(v0, v0, v1, ALU.bitwise_and)
                # bit-plane extraction: plane k = (v >> (k-4)) & 0x1010,
                # one fused shift+mask op each; surviving bit = fp8 2^-5
                pl = planep.tile([128, 8, C2, ehw, 2], FP8, tag="pl")
                pli = pl[:].rearrange("p k c e t -> p (k c e t)").bitcast(I16)
                for k in range(8):
                    dstk = pli[:, k * usz:(k + 1) * usz]
                    if k < 4:
                        nc.vector.tensor_scalar(
                            dstk, v0, 4 - k, 0x1010,
                            ALU.logical_shift_left, ALU.bitwise_and)
                    elif k == 4:
                        nc.vector.tensor_scalar(
                            dstk, v0, 0x1010, 0, ALU.bitwise_and, ALU.bypass)
                    else:
                        nc.vector.tensor_scalar(
                            dstk, v0, k - 4, 0x1010,
                            ALU.logical_shift_right, ALU.bitwise_and)
                # 160 DoubleRow matmuls, x pairs stationary, planes moving:
                # px[c][d, e] += sum_par x[n, 128c+d] * cn[n, e]
                px = pxp.tile([128, NCH, ehw], FP32, tag="px")
                for g in range(NPAIR):
                    k, c2 = g // C2, g % C2
                    rhs = pl[:, k, c2].rearrange("p e t -> p t e")
                    xp = xhl_t[g // prs]
                    lg = g % prs
                    for c in range(NCH):
                        nc.tensor.matmul(
                            px[:, c, :],
                            xp[:, 2 * lg:2 * lg + 2, 128 * c:128 * (c + 1)],
                            rhs, start=(g == 0), stop=(g == NPAIR - 1),
                            perf_mode=DR)
                # xcn^T to bf16 (adding the lo half first when NHALF == 2)
                for t in range(2):
                    dstT = xcnT[:, t, sb * ehw:(sb + 1) * ehw]
                    if NHALF == 2:
                        xcn_sb = workp.tile([128, ehw], FP32, tag="xcn")
                        nc.scalar.activation(xcn_sb[:], px[:, t, :], AF.Copy)
                        nc.vector.tensor_add(dstT, px[:, t + 2, :], xcn_sb[:])
                    else:
                        nc.scalar.activation(dstT, px[:, t, :], AF.Copy)
                mlp_half(sb)


    nc.compile()
    return nc


def _wrap_idx(e_slice, ec):
    """Pack indices for dma_gather: [128, ec//16] int16, idx i at
    [i % 16, i // 16], replicated over the 8 groups of 16 partitions."""
    a = np.asarray(e_slice).astype(np.int16)
    w = a.reshape(ec // 16, 16).T.copy()
    return np.ascontiguousarray(np.tile(w, (8, 1)))


def _plane_row(npad=NPAD):
    """row_of_node[n]: x-table row for node n under the plane-major layout.
    Packed byte m = 256*c2 + 2*r + par of a gathered row lands at partition
    r, and bit k of that byte goes to plane k; the DoubleRow pair for
    (k, c2) contracts par=0,1, so node n = 8*m + k must live at table row
    ((k*C2 + c2)*2 + par)*128 + r."""
    n = np.arange(npad)
    k, m = n % 8, n // 8
    c2, rem = m // 256, m % 256
    r, par = rem // 2, rem % 2
    return ((k * C2 + c2) * 2 + par) * 128 + r


def prepare_inputs(x, adj, edge, W1, b1, W2, b2, W3, b3, Wa, ba, Wb, bb,
                   Wl, bl, beta, n=N, npad=NPAD, ncores=N_CORES):
    x = np.asarray(x, np.float32)
    adj = np.asarray(adj)
    edge = np.asarray(edge)
    ec = edge.shape[0] // ncores

    # bitpacked adjacency, little bit order: byte m bit k = adj[:, 8m+k]
    adjp = np.packbits(adj.astype(bool), axis=1, bitorder="little")
    if adjp.shape[1] < PB:
        adjp = np.pad(adjp, ((0, 0), (0, PB - adjp.shape[1])))
    adjp = np.ascontiguousarray(adjp).view(FP8_NP)

    # fp8 split of 32*x ([hi | lo] when NHALF == 2), rows in plane-major
    # order, then pre-transposed to [partition, group, col] for the load
    xs = XSCALE * x
    x_hi = xs.astype(FP8_NP)
    xhl = np.zeros((npad, XW), FP8_NP)
    row = _plane_row(npad)
    xhl[row[:n], :D] = x_hi
    if NHALF == 2:
        xhl[row[:n], D:] = (xs - x_hi.astype(np.float32)).astype(FP8_NP)
    xhl = np.ascontiguousarray(
        xhl.reshape(npad // 128, 128, XW).transpose(1, 0, 2))

    # bf16 hi/lo split of x, natural row order, for the xi/xj gathers
    xg_hi = x.astype(ml_dtypes.bfloat16)
    xg_lo = (x - xg_hi.astype(np.float32)).astype(ml_dtypes.bfloat16)
    xg = np.zeros((npad, 2 * D), ml_dtypes.bfloat16)
    xg[:n, :D] = xg_hi
    xg[:n, D:] = xg_lo

    # wpack[p, i*512:(i+1)*512] = W_i.T[(k p)] -> [p, (k h)]; +wlt at the end
    BF = ml_dtypes.bfloat16
    bval = np.asarray(beta, np.float32).reshape(-1)[0]
    W3 = np.asarray(W3, np.float32) * bval
    b3 = np.asarray(b3, np.float32) * bval
    wpack = np.zeros((128, 5 * 2 * H + 2), BF)
    for i, W in enumerate((Wa, Wb, W1, W2, W3)):
        wt = np.asarray(W, np.float32).T.astype(BF)      # [D, H], f = 128k + p
        wpack[:, i * 2 * H:(i + 1) * 2 * H] = \
            wt.reshape(2, 128, H).transpose(1, 0, 2).reshape(128, 2 * H)
    wlt = np.asarray(Wl, np.float32).T.astype(BF)        # [H, 1]
    wpack[:, 10 * H:10 * H + 2] = wlt.reshape(2, 128).T
    bpack = np.zeros((128, 11), np.float32)
    for i, b in enumerate((ba, bb, b1, b2, b3)):
        bpack[:, i * 2:(i + 1) * 2] = np.asarray(b, np.float32).reshape(2, 128).T
    bpack[:, 10] = np.asarray(beta, np.float32).reshape(-1)[0]

    common = dict(
        adjp=adjp, xhl=xhl, xg=xg, wpack=wpack, bpack=bpack,
        bl=np.asarray(bl, np.float32).reshape(1, 1),
    )
    in_maps = []
    for c in range(ncores):
        sl = slice(c * ec, (c + 1) * ec)
        m = dict(common)
        m["idx0"] = _wrap_idx(edge[sl, 0], ec)
        m["idx1"] = _wrap_idx(edge[sl, 1], ec)
        in_maps.append(m)
    return in_maps


_CACHE = {}


def _get_program():
    if "nc" not in _CACHE:
        _CACHE["nc"] = build_program()
    return _CACHE["nc"]


def run(in_maps, **kw):
    nc = _get_program()
    return run_bass_kernel_spmd(nc, in_maps, list(range(N_CORES)), **kw)


def kernel(**inputs):
    in_maps = prepare_inputs(**inputs)
    res = run(in_maps)
    out = np.concatenate([res.results[c]["out"][0] for c in range(N_CORES)])
    return out.reshape(E, 1).astype(np.float32)


# revision 22
# speedup vs baseline: 1.0098x; 1.0098x over previous
"""CNLP (common-neighbor link prediction) kernel for Trainium2, 8 NeuronCores.

Reference computation (per query edge e = (i, j)):
    cn  = adj[i] * adj[j]                      # common-neighbor indicator [N]
    xcn = cn @ x                               # sum of common-neighbor feats
    xij = relu(x[i]*x[j] @ Wa.T + ba) @ Wb.T + bb
    hcn = (relu->relu->lin) 3-layer MLP on xcn
    out = (hcn * beta + xij) @ Wl.T + bl       # [E, 1]

Sharding: edges (E=8192) split 8 x 1024 across cores; adj/x/weights replicated.

Device strategy per core (v2 — bitpacked adjacency + fp8 DoubleRow):
  - adj rows are BITPACKED on the host: [N, 1280] bytes (8 nodes/byte,
    little bit order).  gpsimd dma_gather(transpose=True) pulls the two
    packed rows per edge batch (128 edges) — 8x less HBM traffic than fp8.
  - DVE ANDs the packed rows (bitwise AND == product for binary rows),
    then extracts 8 bit-planes with one fused shift+mask tensor_scalar op
    per plane: each surviving bit sits at position 4 of its byte, i.e.
    fp8e4m3 value 2^-5; the matching x-table rows are pre-scaled by 2^5.
  - x is uploaded as a [10240, 512] fp8 table of [x_hi | x_lo] (hi/lo split
    of 32*x, so two e4m3 matmuls recover ~9-bit-mantissa accuracy), rows
    permuted to plane-major order so each (plane, byte-chunk) pair of
    128-node groups is contiguous.
  - TensorE runs fp8 DoubleRow matmuls (256-deep contraction per
    instruction, 0.5 PE cycles/row) with the x-table pairs as the
    STATIONARY operand (contiguous, satisfies the dual-fp8 Ldweights
    stride rules) and the cn planes as the moving operand: the output
    lands feature-major ([d, e] = xcn^T) so no PE transposes are needed;
    ACT+DVE fold the hi/lo halves while converting to bf16.
  - xi/xj are gathered from a separate natural-order bf16 [x_hi | x_lo]
    table (exact same path as v1, proven on HW).
  - MLPs run in bf16 (1 PE cycle/row), feature-major, biases+ReLU fused
    into ScalarE activations.  Output [1, 1024] fp32 per core.
"""

import numpy as np
import ml_dtypes

import concourse.bacc as bacc
import concourse.tile as tile
import concourse.mybir as mybir
from concourse.bass_utils import run_bass_kernel_spmd

BF16 = mybir.dt.bfloat16
FP32 = mybir.dt.float32
F32R = mybir.dt.float32r
FP8 = mybir.dt.float8e4
I16 = mybir.dt.int16
AF = mybir.ActivationFunctionType
ALU = mybir.AluOpType
DR = mybir.MatmulPerfMode.DoubleRow
FP8_NP = ml_dtypes.float8_e4m3

N_CORES = 8
N, E, D, H = 10000, 8192, 256, 256
NPAD = 10240                      # n padded to a multiple of 2048
PB = NPAD // 8                    # packed bytes per adjacency row (1280)
C2 = PB // 256                    # 256-byte chunks per row (5)
NPAIR = 8 * C2                    # DoubleRow matmuls per edge batch (40)
EC = E // N_CORES                 # 1024 edges per core
EB = 128                          # edges per batch (one gather tile)
XSCALE = 32.0                     # x table pre-scale (2^5, exact in fp8)
NHALF = 1                         # 1 = single fp8 x table, 2 = [hi | lo]
XW = D * NHALF                    # fp8 x-table row width
NCH = XW // 128                   # output column chunks (2 or 4)


def build_program(npad=NPAD, ec=EC):
    nb = ec // EB                  # edge batches (8)
    ehw = min(512, ec)             # edge columns per MLP matmul
    n_eh = ec // ehw               # MLP halves (2)
    n_xs = 8                       # x table sub-tiles
    prs = NPAIR // n_xs            # pairs per x sub-tile (5)
    u16 = PB // 2                  # int16 lanes per packed row (640)

    nc = bacc.Bacc("TRN2", target_bir_lowering=False, debug=False,
                   enable_asserts=False, num_devices=N_CORES)

    adjp_d = nc.dram_tensor("adjp", [N, PB], FP8, kind="ExternalInput")
    xhl_d = nc.dram_tensor("xhl", [128, npad // 128, XW], FP8,
                           kind="ExternalInput")
    xg_d = nc.dram_tensor("xg", [npad, 2 * D], BF16, kind="ExternalInput")
    # gather indices: raw node ids (adjacency rows + natural-order xg rows)
    idx0_d = nc.dram_tensor("idx0", [128, ec // 16], I16, kind="ExternalInput")
    idx1_d = nc.dram_tensor("idx1", [128, ec // 16], I16, kind="ExternalInput")
    # all MLP weights pre-transposed to [partition, k-chunk, h] and packed
    # into one tensor (one contiguous descriptor per partition); biases+beta
    # likewise packed as fp32
    wpack_d = nc.dram_tensor("wpack", [128, 5 * 2 * H + 2], BF16,
                             kind="ExternalInput")
    bpack_d = nc.dram_tensor("bpack", [128, 11], FP32, kind="ExternalInput")
    bl_d = nc.dram_tensor("bl", [1, 1], FP32, kind="ExternalInput")
    out_d = nc.dram_tensor("out", [1, ec], FP32, kind="ExternalOutput")

    with tile.TileContext(nc) as tc:
        with (
            tc.tile_pool(name="const", bufs=1) as constp,
            tc.tile_pool(name="gath", bufs=1) as gathp,
            tc.tile_pool(name="plane", bufs=2) as planep,
            tc.tile_pool(name="work", bufs=2) as workp,
            tc.tile_pool(name="ghl", bufs=1) as ghlp,
            tc.tile_pool(name="acts", bufs=4) as actp,
            tc.tile_pool(name="px", bufs=3 - NHALF, space="PSUM") as pxp,
            tc.tile_pool(name="pm", bufs=2, space="PSUM") as pmp,
            tc.tile_pool(name="po", bufs=2, space="PSUM") as pop,
        ):
            # ---- persistent loads -------------------------------------
            # idx tiles FIRST: every gather waits on them, and HWDGE DMAs
            # execute in FIFO order per issuing engine
            idx0_sb = constp.tile([128, ec // 16], I16)
            nc.sync.dma_start(idx0_sb[:], idx0_d[:])
            idx1_sb = constp.tile([128, ec // 16], I16)
            nc.sync.dma_start(idx1_sb[:], idx1_d[:])

            # whole x table in one DMA (pre-transposed on the host: one
            # contiguous 20KB descriptor per partition)
            xall = constp.tile([128, NPAIR * 2, XW], FP8)
            nc.sync.dma_start(xall[:], xhl_d[:])
            xhl_t = [xall[:, i * 2 * prs:(i + 1) * 2 * prs, :]
                     for i in range(n_xs)]

            wpack = constp.tile([128, 5 * 2 * H + 2], BF16)
            nc.sync.dma_start(wpack[:], wpack_d[:])
            w_sb = {nm: wpack[:, i * 2 * H:(i + 1) * 2 * H]
                    .rearrange("p (k h) -> p k h", k=2)
                    for i, nm in enumerate(("wat", "wbt", "w1t", "w2t", "w3t"))}
            wlt_sb = wpack[:, 10 * H:10 * H + 2].rearrange(
                "p (k o) -> p k o", k=2)
            bpack = constp.tile([128, 11], FP32)
            nc.sync.dma_start(bpack[:], bpack_d[:])
            b_sb = {nm: bpack[:, i * 2:(i + 1) * 2]
                    .rearrange("p (t o) -> p t o", t=2)
                    for i, nm in enumerate(("ba", "bb", "b1", "b2", "b3"))}
            beta_sb = bpack[:, 10:11]
            bl_sb = constp.tile([1, 1], FP32)
            nc.sync.dma_start(bl_sb[:], bl_d[:])

            # hoist every gather: descriptor generation serializes on the
            # gpsimd engine, so issue all eight back-to-back up front
            gt = {}
            for s in range(n_eh):
                isl = slice(s * (ehw // 16), (s + 1) * (ehw // 16))
                for nm, isb in (("g0", idx0_sb), ("g1", idx1_sb)):
                    g = gathp.tile([128, PB // 128, ehw], FP8,
                                   tag=f"{nm}_{s}")
                    nc.gpsimd.dma_gather(g[:], adjp_d[:], isb[:, isl],
                                         ehw, ehw, elem_size=PB,
                                         transpose=True)
                    gt[nm, s] = g
            ghl_t = {}
            for s in range(n_eh):
                isl = slice(s * (ehw // 16), (s + 1) * (ehw // 16))
                for nm, isb in (("xi", idx0_sb), ("xj", idx1_sb)):
                    ghl = ghlp.tile([128, 4, ehw], BF16, tag=f"ghl{nm}{s}")
                    nc.gpsimd.dma_gather(ghl[:], xg_d[:], isb[:, isl],
                                         ehw, ehw, elem_size=2 * D,
                                         transpose=True)
                    ghl_t[nm, s] = ghl

            xcnT = constp.tile([128, 2, ec], BF16)   # feature-major xcn
            out_sb = constp.tile([1, ec], FP32)

            # ---- MLP for one edge-half (ehw edges), feature-major f32r;
            # emitted mid-loop so its PE work overlaps gather-bound spans.
            def lin_h(src_ap, wname, bname, relu, dst):
                w, bias = w_sb[wname], b_sb[bname]
                for t in range(2):
                    pm = pmp.tile([128, ehw], FP32, tag="pm")
                    for k in range(2):
                        nc.tensor.matmul(
                            pm[:], w[:, k, t * 128:(t + 1) * 128],
                            src_ap[:, k, :], start=(k == 0), stop=(k == 1))
                    dsl = dst[:, t, :]
                    if t % 2 == 0:
                        nc.scalar.activation(
                            dsl, pm[:], AF.Relu if relu else AF.Identity,
                            bias=bias[:, t, :])
                    elif relu:
                        nc.vector.tensor_scalar(
                            dsl, pm[:], bias[:, t, :], 0.0, ALU.add, ALU.max)
                    else:
                        nc.vector.tensor_scalar_add(dsl, pm[:], bias[:, t, :])
                return dst

            def mlp_half(hh):
                esl = slice(hh * ehw, (hh + 1) * ehw)
                xiT = actp.tile([128, 2, ehw], FP32, tag="act")
                xjT = actp.tile([128, 2, ehw], FP32, tag="act")
                for dst, nm in ((xiT, "xi"), (xjT, "xj")):
                    ghl = ghl_t[nm, hh]
                    nc.vector.tensor_add(dst[:], ghl[:, 0:2, :], ghl[:, 2:4, :])
                pT = actp.tile([128, 2, ehw], BF16, tag="actb")
                nc.vector.tensor_mul(pT[:], xiT[:], xjT[:])
                u = lin_h(pT[:], "wat", "ba", True,
                          actp.tile([128, 2, ehw], BF16, tag="actb",
                                    name=f"u{hh}"))
                xijT = lin_h(u[:], "wbt", "bb", False,
                             actp.tile([128, 2, ehw], BF16, tag="actb",
                                       name=f"xijT{hh}"))
                h = xcnT[:, :, esl]
                for li, (wn, bn, rl) in enumerate((
                        ("w1t", "b1", True), ("w2t", "b2", True),
                        ("w3t", "b3", False))):
                    h = lin_h(h, wn, bn, rl,
                              actp.tile([128, 2, ehw], BF16, tag="actb",
                                        name=f"h{hh}_{li}"))[:]
                nc.vector.tensor_scalar_mul(h, h, beta_sb[:])
                nc.vector.tensor_add(h, h, xijT[:])
                po = pop.tile([1, ehw], FP32, tag="po")
                for k in range(2):
                    nc.tensor.matmul(po[:], wlt_sb[:, k, :], h[:, k, :],
                                     start=(k == 0), stop=(k == 1))
                nc.scalar.activation(out_sb[:, hh * ehw:(hh + 1) * ehw],
                                     po[:], AF.Identity, bias=bl_sb[:])

            # ---- main loop: packed gather -> AND -> planes -> xcn^T ---
            for sb in range(n_eh):
                g0, g1 = gt["g0", sb], gt["g1", sb]
                # cn = g0 AND g1 on int16 lanes (two packed bytes per lane)
                v0 = g0[:].rearrange("p a b -> p (a b)").bitcast(I16)
                v1 = g1[:].rearrange("p a b -> p (a b)").bitcast(I16)
                usz = PB * ehw // 256
                nc.vector.tensor_tensor(v0, v0, v1, ALU.bitwise_and)
                # bit-plane extraction: plane k = (v >> (k-4)) & 0x1010,
                # one fused shift+mask op each; surviving bit = fp8 2^-5
                pl = planep.tile([128, 8, C2, ehw, 2], FP8, tag="pl")
                pli = pl[:].rearrange("p k c e t -> p (k c e t)").bitcast(I16)
                for k in range(8):
                    dstk = pli[:, k * usz:(k + 1) * usz]
                    if k < 4:
                        nc.vector.tensor_scalar(
                            dstk, v0, 4 - k, 0x1010,
                            ALU.logical_shift_left, ALU.bitwise_and)
                    elif k == 4:
                        nc.vector.tensor_scalar(
                            dstk, v0, 0x1010, 0, ALU.bitwise_and, ALU.bypass)
                    else:
                        nc.vector.tensor_scalar(
                            dstk, v0, k - 4, 0x1010,
                            ALU.logical_shift_right, ALU.bitwise_and)
                # 160 DoubleRow matmuls, x pairs stationary, planes moving:
                # px[c][d, e] += sum_par x[n, 128c+d] * cn[n, e]
                px = pxp.tile([128, NCH, ehw], FP32, tag="px")
                for g in range(NPAIR):
                    k, c2 = g // C2, g % C2
                    rhs = pl[:, k, c2].rearrange("p e t -> p t e")
                    xp = xhl_t[g // prs]
                    lg = g % prs
                    for c in range(NCH):
                        nc.tensor.matmul(
                            px[:, c, :],
                            xp[:, 2 * lg:2 * lg + 2, 128 * c:128 * (c + 1)],
                            rhs, start=(g == 0), stop=(g == NPAIR - 1),
                            perf_mode=DR)
                # xcn^T to bf16 (adding the lo half first when NHALF == 2)
                for t in range(2):
                    dstT = xcnT[:, t, sb * ehw:(sb + 1) * ehw]
                    if NHALF == 2:
                        xcn_sb = workp.tile([128, ehw], FP32, tag="xcn")
                        nc.scalar.activation(xcn_sb[:], px[:, t, :], AF.Copy)
                        nc.vector.tensor_add(dstT, px[:, t + 2, :], xcn_sb[:])
                    else:
                        nc.scalar.activation(dstT, px[:, t, :], AF.Copy)
                mlp_half(sb)

            nc.sync.dma_start(out_d[:], out_sb[:])

    nc.compile()
    return nc


def _wrap_idx(e_slice, ec):
    """Pack indices for dma_gather: [128, ec//16] int16, idx i at
    [i % 16, i // 16], replicated over the 8 groups of 16 partitions."""
    a = np.asarray(e_slice).astype(np.int16)
    w = a.reshape(ec // 16, 16).T.copy()
    return np.ascontiguousarray(np.tile(w, (8, 1)))


def _plane_row(npad=NPAD):
    """row_of_node[n]: x-table row for node n under the plane-major layout.
    Packed byte m = 256*c2 + 2*r + par of a gathered row lands at partition
    r, and bit k of that byte goes to plane k; the DoubleRow pair for
    (k, c2) contracts par=0,1, so node n = 8*m + k must live at table row
    ((k*C2 + c2)*2 + par)*128 + r."""
    n = np.arange(npad)
    k, m = n % 8, n // 8
    c2, rem = m // 256, m % 256
    r, par = rem // 2, rem % 2
    return ((k * C2 + c2) * 2 + par) * 128 + r


def prepare_inputs(x, adj, edge, W1, b1, W2, b2, W3, b3, Wa, ba, Wb, bb,
                   Wl, bl, beta, n=N, npad=NPAD, ncores=N_CORES):
    x = np.asarray(x, np.float32)
    adj = np.asarray(adj)
    edge = np.asarray(edge)
    ec = edge.shape[0] // ncores

    # bitpacked adjacency, little bit order: byte m bit k = adj[:, 8m+k]
    adjp = np.packbits(adj.astype(bool), axis=1, bitorder="little")
    if adjp.shape[1] < PB:
        adjp = np.pad(adjp, ((0, 0), (0, PB - adjp.shape[1])))
    adjp = np.ascontiguousarray(adjp).view(FP8_NP)

    # fp8 split of 32*x ([hi | lo] when NHALF == 2), rows in plane-major
    # order, then pre-transposed to [partition, group, col] for the load
    xs = XSCALE * x
    x_hi = xs.astype(FP8_NP)
    xhl = np.zeros((npad, XW), FP8_NP)
    row = _plane_row(npad)
    xhl[row[:n], :D] = x_hi
    if NHALF == 2:
        xhl[row[:n], D:] = (xs - x_hi.astype(np.float32)).astype(FP8_NP)
    xhl = np.ascontiguousarray(
        xhl.reshape(npad // 128, 128, XW).transpose(1, 0, 2))

    # bf16 hi/lo split of x, natural row order, for the xi/xj gathers
    xg_hi = x.astype(ml_dtypes.bfloat16)
    xg_lo = (x - xg_hi.astype(np.float32)).astype(ml_dtypes.bfloat16)
    xg = np.zeros((npad, 2 * D), ml_dtypes.bfloat16)
    xg[:n, :D] = xg_hi
    xg[:n, D:] = xg_lo

    # wpack[p, i*512:(i+1)*512] = W_i.T[(k p)] -> [p, (k h)]; +wlt at the end
    BF = ml_dtypes.bfloat16
    wpack = np.zeros((128, 5 * 2 * H + 2), BF)
    for i, W in enumerate((Wa, Wb, W1, W2, W3)):
        wt = np.asarray(W, np.float32).T.astype(BF)      # [D, H], f = 128k + p
        wpack[:, i * 2 * H:(i + 1) * 2 * H] = \
            wt.reshape(2, 128, H).transpose(1, 0, 2).reshape(128, 2 * H)
    wlt = np.asarray(Wl, np.float32).T.astype(BF)        # [H, 1]
    wpack[:, 10 * H:10 * H + 2] = wlt.reshape(2, 128).T
    bpack = np.zeros((128, 11), np.float32)
    for i, b in enumerate((ba, bb, b1, b2, b3)):
        bpack[:, i * 2:(i + 1) * 2] = np.asarray(b, np.float32).reshape(2, 128).T
    bpack[:, 10] = np.asarray(beta, np.float32).reshape(-1)[0]

    common = dict(
        adjp=adjp, xhl=xhl, xg=xg, wpack=wpack, bpack=bpack,
        bl=np.asarray(bl, np.float32).reshape(1, 1),
    )
    in_maps = []
    for c in range(ncores):
        sl = slice(c * ec, (c + 1) * ec)
        m = dict(common)
        m["idx0"] = _wrap_idx(edge[sl, 0], ec)
        m["idx1"] = _wrap_idx(edge[sl, 1], ec)
        in_maps.append(m)
    return in_maps


_CACHE = {}


def _get_program():
    if "nc" not in _CACHE:
        _CACHE["nc"] = build_program()
    return _CACHE["nc"]


def run(in_maps, **kw):
    nc = _get_program()
    return run_bass_kernel_spmd(nc, in_maps, list(range(N_CORES)), **kw)


def kernel(**inputs):
    in_maps = prepare_inputs(**inputs)
    res = run(in_maps)
    out = np.concatenate([res.results[c]["out"][0] for c in range(N_CORES)])
    return out.reshape(E, 1).astype(np.float32)
